# revision 1
# baseline (speedup 1.0000x reference)
"""Trainium2 Bass kernel for nn_MultiHeadAttention (B=2, S=4096, D=768, H=12, DH=64).

Sharding: 8 cores = 2 batches x 4 head-groups (3 heads each).
Each core computes its heads' attention for its batch and a partial
out^T = Wo_slice^T.T @ att^T ; host sums the 4 head-group partials per batch.

Mask trick: key positions with att_mask==1 are dropped on the host
(K/V computed only over kept positions, padded to a multiple of 128 with
zero columns). Pad columns give scores exactly 0 -> exp = 1, corrected by
Z -= n_pad. Pad V rows are zero so U is unaffected.
"""

import math

import numpy as np
import ml_dtypes

import concourse.bass as bass
import concourse.mybir as mybir
import concourse.tile as tile
from concourse import bacc
from concourse.bass_utils import run_bass_kernel_spmd

# ---------------- custom DVE exp2 op (bf16 bits of 2^(x/128)) ----------------
import concourse.dve_ops as _dve_ops
from concourse.dve_spec import (
    Spec as _Spec, Src0 as _Src0, C0 as _C0, C1 as _C1, C2 as _C2, C3 as _C3,
    Bin as _Bin, _spill_c3_to_src1 as _spill, lower as _dve_lower,
    _has_src1 as _dve_has_src1,
)
from concourse.dve_uop import AluOp as _AluOp, DveOpSpec as _DveOpSpec

EXP2_A128 = 44.234904699532095 / (128.0 * 128.0)
EXP2_B128 = -44.03494444066827 / 128.0
EXP2_MAGIC = float(np.float32(1.5 * 2.0**30))
EXP2_G = 1.9938757033212957  # effective output scale (trunc store, sim-measured)
SCORE_PRESCALE = 128.0 * np.log2(np.e) / 8.0  # folded into Wq/bq on host


def _exp2_ref(in0, in1, s0, s1, imm2):
    x = np.asarray(in0, np.float32)
    b = np.float32(np.asarray(in1).reshape(-1)[0]) if np.ndim(in1) else np.float32(in1)
    r = (x + np.float32(s1)) - np.float32(s1)
    F = np.abs(x - r)
    q = F * (np.float32(s0) * F + b)
    return q + (x + np.float32(imm2) * np.float32(imm2))


def _register_exp2_op():
    if "EXP2B_ANT" in _dve_ops._SUB_OPCODE_FOR_NAME:
        for _op in _dve_ops.OPS:
            if _op.name == "EXP2B_ANT":
                return _op
    K = _Bin(_AluOp.MULTIPLY, _C2, _C2)
    t = _Src0 + _C1
    r = t - _C1
    F = _Bin(_AluOp.ABSOLUTE_DIFF, _Src0, r)
    q = _Bin(_AluOp.MULTIPLY, F, _C0 * F + _C3)
    body = q + (_Src0 + K)
    spec = _Spec(body=_spill(body), reference=_exp2_ref)
    shas = {}
    for ver in ("v3", "v4"):
        try:
            tmp = _DveOpSpec(
                name="EXP2B_ANT", opcode=17, uops=_dve_lower(spec, ver=ver),
                rd1_en=_dve_has_src1(spec),
            )
            shas[ver] = tmp.sha(ver)
        except Exception:
            pass
    op = _dve_ops.DveOp("EXP2B_ANT", spec, False, shas)
    _dve_ops.OPS.append(op)
    _dve_ops.CUSTOM_DVE_SPECS[op.name] = op.spec
    _dve_ops._SUB_OPCODE_FOR_NAME[op.name] = 17
    return op


EXP2_OP = _register_exp2_op()


class CachedRunner:
    """Compile the Bass program into a PJRT executable once; reuse across calls."""

    def __init__(self, nc, n_cores=8):
        import jax
        from jax.sharding import Mesh, PartitionSpec
        from jax.experimental.shard_map import shard_map as _shard_map
        from concourse.bass2jax import (
            _bass_exec_p,
            install_neuronx_cc_hook,
            partition_id_tensor,
        )
        import concourse.mybir as _mybir

        install_neuronx_cc_hook()
        self.nc = nc
        self.n_cores = n_cores
        part_name = nc.partition_id_tensor.name if nc.partition_id_tensor else None
        in_names, out_names, out_avals, zero_shapes = [], [], [], []
        for alloc in nc.m.functions[0].allocations:
            if not isinstance(alloc, _mybir.MemoryLocationSet):
                continue
            name = alloc.memorylocations[0].name
            if alloc.kind == "ExternalInput":
                if name != part_name:
                    in_names.append(name)
            elif alloc.kind == "ExternalOutput":
                out_names.append(name)
                shape = tuple(alloc.tensor_shape)
                dtype = _mybir.dt.np(alloc.dtype)
                out_avals.append(jax.core.ShapedArray(shape, dtype))
                zero_shapes.append((shape, dtype))
        self.in_names, self.out_names = in_names, out_names
        self.out_avals = out_avals
        self.zero_shapes = zero_shapes
        n_params = len(in_names)
        all_in_names = tuple(in_names) + tuple(out_names)
        if part_name is not None:
            all_in_names = all_in_names + (part_name,)

        def _body(*args):
            operands = list(args)
            if part_name is not None:
                operands.append(partition_id_tensor())
            return tuple(
                _bass_exec_p.bind(
                    *operands,
                    out_avals=tuple(out_avals),
                    in_names=all_in_names,
                    out_names=tuple(out_names),
                    lowering_input_output_aliases=(),
                    sim_require_finite=True,
                    sim_require_nnan=True,
                    nc=nc,
                )
            )

        devices = jax.devices()[:n_cores]
        mesh = Mesh(np.asarray(devices), ("core",))
        nin = n_params + len(out_names)
        self._fn = jax.jit(
            _shard_map(
                _body,
                mesh=mesh,
                in_specs=(PartitionSpec("core"),) * nin,
                out_specs=(PartitionSpec("core"),) * len(out_names),
                check_rep=False,
            ),
            keep_unused=True,
        )
        self._jax = jax

    def __call__(self, in_maps):
        jax = self._jax
        concat = [
            np.concatenate([np.asarray(m[name]) for m in in_maps], axis=0)
            for name in self.in_names
        ]
        zeros = [
            np.zeros((self.n_cores * s[0],) + tuple(s[1:]), d)
            for s, d in self.zero_shapes
        ]
        outs = self._fn(*concat, *zeros)
        jax.block_until_ready(outs)
        res = []
        for c in range(self.n_cores):
            m = {}
            for i, name in enumerate(self.out_names):
                shape = self.out_avals[i].shape
                m[name] = np.asarray(outs[i]).reshape((self.n_cores,) + shape)[c]
            res.append(m)
        return res

B, S, D, H, DH = 2, 4096, 768, 12, 64
HPC = 3  # heads per core
NCORES = 8
EC = D // 128  # 6 e-chunks
QT = 512  # q tile (psum bank)
QG = 1024  # q group width for the exp ACT batch
BF16 = ml_dtypes.bfloat16

_prog_cache: dict = {}


def _kchunks(total, step):
    out = []
    o = 0
    while o < total:
        w = min(step, total - o)
        out.append((o, w))
        o += w
    return out


def build_program(Skc: int, s_full: int = S, repeat: int = 1, pair: bool = True, prio: int = 700):
    """Build the per-core Bass program. Skc = padded kept-key count (mult of 128)."""
    f32 = mybir.dt.float32
    bf16 = mybir.dt.bfloat16
    nkc = Skc // 128
    dve_kcs = {kc for kc in range(nkc) if kc % 3 == 1 and kc < nkc - 2}
    n_qt = s_full // QT
    qg_w = min(QG, s_full)
    nqi = qg_w // QT
    n_qg = s_full // qg_w

    nc = bacc.Bacc()
    hT = nc.dram_tensor("hT", [D, s_full], bf16, kind="ExternalInput")
    hTk = nc.dram_tensor("hTk", [D, Skc], bf16, kind="ExternalInput")
    wq = nc.dram_tensor("wq", [D, HPC * DH], bf16, kind="ExternalInput")
    wk = nc.dram_tensor("wk", [D, HPC * DH], bf16, kind="ExternalInput")
    wv = nc.dram_tensor("wv", [D, HPC * DH], bf16, kind="ExternalInput")
    wo = nc.dram_tensor("wo", [HPC * DH, D], bf16, kind="ExternalInput")
    bq = nc.dram_tensor("bq", [HPC * DH, 1], f32, kind="ExternalInput")
    bk = nc.dram_tensor("bk", [HPC * DH, 1], f32, kind="ExternalInput")
    bv = nc.dram_tensor("bv", [1, HPC * DH], bf16, kind="ExternalInput")
    npad = nc.dram_tensor("npad", [1, 1], f32, kind="ExternalInput")
    out = nc.dram_tensor("out", [D, s_full], f32, kind="ExternalOutput")

    Exp = mybir.ActivationFunctionType.Exp

    import contextlib
    with tile.TileContext(nc) as tc, contextlib.ExitStack() as _stk:
        _pp = _stk.enter_context(tc.tile_pool(name="persist", bufs=1))
        def _tctile(shape, dtype, name):
            return _pp.tile(shape, dtype, name=name, tag=name)
        # ---- persistent SBUF tiles ----
        hT_sb = _tctile([128, EC, s_full], bf16, name="hT_sb")
        hTk_sb = _tctile([128, EC, Skc], bf16, name="hTk_sb")
        wq_sb = _tctile([128, EC, HPC * DH], bf16, name="wq_sb")
        wk_sb = _tctile([128, EC, HPC * DH], bf16, name="wk_sb")
        wv_sb = _tctile([128, EC, HPC * DH], bf16, name="wv_sb")
        wo_sb_t = _tctile([128, HPC, D], bf16, name="wo_sb")
        wo_sb = wo_sb_t[0:DH, :, :]
        bqp_sb = _tctile([128, 1], f32, name="bqp_sb")
        bq2_sb_t = _tctile([128, 1], f32, name="bq2_sb")
        bq2_sb = bq2_sb_t[0:DH, :]
        bkp_sb = _tctile([128, 1], f32, name="bkp_sb")
        bk2_sb_t = _tctile([128, 1], f32, name="bk2_sb")
        bk2_sb = bk2_sb_t[0:DH, :]
        bv_sb_t = _tctile([128, HPC * DH], bf16, name="bv_sb")
        bv_sb = bv_sb_t[0:1, :]
        eb1_sb = _tctile([128, 1], f32, name="eb1_sb")
        abias_sb = _tctile([128, 1], f32, name="abias_sb")
        npad_sb_t = _tctile([128, 1], f32, name="npad_sb")
        npad_sb = npad_sb_t[0:1, :]
        ones_f_t = _tctile([128, DH], f32, name="ones_f")
        ones_f = ones_f_t[0:1, :]
        ones_b_t = _tctile([128, 128], bf16, name="ones_b")
        ones_b = ones_b_t[0:1, :]

        qp_t = [_tctile([128, QT], bf16, name=f"qp{i}") for i in range(n_qt)]
        qp2_t = [_tctile([128, QT], bf16, name=f"qp2_{i}") for i in range(n_qt)]
        q2_t = [_tctile([128, QT], bf16, name=f"q2_{i}") for i in range(n_qt)]
        nkch = len(_kchunks(Skc, QT))
        kp_t = [_tctile([128, QT], bf16, name=f"kp{i}") for i in range(nkch)]
        kp2_t = [_tctile([128, QT], bf16, name=f"kp2_{i}") for i in range(nkch)]
        k2_t = [_tctile([128, QT], bf16, name=f"k2_{i}") for i in range(nkch)]
        v_t = [_tctile([128, HPC, DH + 1], bf16, name=f"v{sc}") for sc in range(nkc)]
        un_t = [
            [_tctile([128, QT], bf16, name=f"un{h}_{i}") for i in range(n_qt)]
            for h in range(HPC)
        ]

        # input DMAs (small weights first so projections can start immediately)
        nc.sync.dma_start(out=wq_sb, in_=wq[:, :].rearrange("(c p) m -> p c m", p=128))
        nc.sync.dma_start(out=wk_sb, in_=wk[:, :].rearrange("(c p) m -> p c m", p=128))
        nc.sync.dma_start(out=wv_sb, in_=wv[:, :].rearrange("(c p) m -> p c m", p=128))
        nc.sync.dma_start(out=wo_sb, in_=wo[:, :].rearrange("(h d) e -> d h e", d=DH))
        nc.sync.dma_start(out=bqp_sb, in_=bq[0:128, :])
        nc.sync.dma_start(out=bq2_sb, in_=bq[128 : HPC * DH, :])
        nc.sync.dma_start(out=bkp_sb, in_=bk[0:128, :])
        nc.sync.dma_start(out=bk2_sb, in_=bk[128 : HPC * DH, :])
        nc.sync.dma_start(out=bv_sb, in_=bv[:, :])
        nc.sync.dma_start(out=npad_sb, in_=npad[:, :])
        for ec in range(EC):
            nc.sync.dma_start(
                out=hT_sb[:, ec, :], in_=hT[bass.ts(ec, 128), :]
            )
            nc.sync.dma_start(
                out=hTk_sb[:, ec, :], in_=hTk[bass.ts(ec, 128), :]
            )
        nc.vector.memset(ones_f, 1.0)
        nc.vector.memset(eb1_sb, EXP2_B128)
        nc.vector.memset(abias_sb, float(np.log(EXP2_G)))
        nc.vector.memset(ones_b, 1.0)
        for sc in range(nkc):
            nc.vector.memset(v_t[sc][:, :, DH : DH + 1], 1.0)

        def body(_iv=None):
            import contextlib as _cl
            with _cl.ExitStack() as stk:
                pp = stk.enter_context(tc.tile_pool(name="ps", bufs=1, space="PSUM"))
                esb = stk.enter_context(tc.tile_pool(name="esb", bufs=3))
                zsb = stk.enter_context(tc.tile_pool(name="zsb", bufs=2))
                obp = stk.enter_context(tc.tile_pool(name="ob", bufs=3))

                # ---- projection emitters (psum slots shared with scores, tag "s") ----
                def q_proj(qi):
                    qsl = bass.ts(qi, QT)
                    ps = pp.tile([128, qg_w], f32, tag="s", bufs=2, name="ps")
                    for ec in range(EC):
                        nc.tensor.matmul(
                            ps[:, 0:QT], wq_sb[:, ec, 0:128], hT_sb[:, ec, qsl],
                            start=(ec == 0), stop=(ec == EC - 1),
                        )
                    for ec in range(EC):
                        nc.tensor.matmul(
                            ps[0:DH, QT : QT + QT], wq_sb[:, ec, 128 : HPC * DH],
                            hT_sb[:, ec, qsl],
                            start=(ec == 0), stop=(ec == EC - 1),
                        )
                    nc.vector.tensor_scalar_add(qp_t[qi][:, :], ps[:, 0:QT], bqp_sb)
                    nc.vector.tensor_scalar_add(q2_t[qi][0:DH, :], ps[0:DH, QT : QT + QT], bq2_sb)
                    nc.sync.dma_start(out=qp2_t[qi][0:64, :], in_=qp_t[qi][64:128, :])
                    nc.sync.dma_start(out=qp2_t[qi][64:128, :], in_=qp_t[qi][0:64, :])
                    nc.sync.dma_start(out=q2_t[qi][64:128, :], in_=q2_t[qi][0:64, :])

                def k_proj(i, off, w):
                    ksl = bass.ds(off, w)
                    ps = pp.tile([128, qg_w], f32, tag="s", bufs=2, name="ps")
                    for ec in range(EC):
                        nc.tensor.matmul(
                            ps[:, 0:w], wk_sb[:, ec, 0:128], hTk_sb[:, ec, ksl],
                            start=(ec == 0), stop=(ec == EC - 1),
                        )
                    for ec in range(EC):
                        nc.tensor.matmul(
                            ps[0:DH, QT : QT + w], wk_sb[:, ec, 128 : HPC * DH],
                            hTk_sb[:, ec, ksl],
                            start=(ec == 0), stop=(ec == EC - 1),
                        )
                    nc.vector.tensor_scalar_add(kp_t[i][:, 0:w], ps[:, 0:w], bkp_sb)
                    nc.vector.tensor_scalar_add(k2_t[i][0:DH, 0:w], ps[0:DH, QT : QT + w], bk2_sb)
                    nc.sync.dma_start(out=kp2_t[i][0:64, 0:w], in_=kp_t[i][64:128, 0:w])
                    nc.sync.dma_start(out=kp2_t[i][64:128, 0:w], in_=kp_t[i][0:64, 0:w])
                    nc.sync.dma_start(out=k2_t[i][64:128, 0:w], in_=k2_t[i][0:64, 0:w])

                def v_proj(sc):
                    psv = pp.tile([128, qg_w], f32, tag="s", bufs=2, name="psv")
                    for ec in range(EC):
                        nc.tensor.matmul(
                            psv[:, 0 : HPC * DH], hTk_sb[:, ec, bass.ts(sc, 128)],
                            wv_sb[:, ec, :],
                            start=(ec == 0), stop=False,
                        )
                    nc.tensor.matmul(
                        psv[:, 0 : HPC * DH], ones_b[0:1, :], bv_sb,
                        start=False, stop=True,
                    )
                    nc.vector.tensor_copy(
                        v_t[sc][:, :, 0:DH],
                        psv[:, 0 : HPC * DH].rearrange("p (h d) -> p h d", d=DH),
                    )

                # emit projections: enough for attention to start, rest follows
                for qi in range(nqi):
                    q_proj(qi)
                kch = _kchunks(Skc, QT)
                for i, (off, w) in enumerate(kch):
                    k_proj(i, off, w)
                    for sc in range(4 * i, min(4 * (i + 1), nkc)):
                        v_proj(sc)
                for sc in range(4 * len(kch), nkc):
                    v_proj(sc)
                for qi in range(nqi, n_qt):
                    q_proj(qi)

                # ---- flat pipelined attention stream ----
                def head_views(h):
                    if h == 0:
                        return (
                            lambda qi: qp_t[qi][0:64, :], lambda qi: qp2_t[qi][64:128, :],
                            lambda i: kp_t[i][0:64, :], lambda i: kp2_t[i][64:128, :],
                        )
                    if h == 1:
                        return (
                            lambda qi: qp2_t[qi][0:64, :], lambda qi: qp_t[qi][64:128, :],
                            lambda i: kp2_t[i][0:64, :], lambda i: kp_t[i][64:128, :],
                        )
                    return (
                        lambda qi: q2_t[qi][0:64, :], lambda qi: q2_t[qi][64:128, :],
                        lambda i: k2_t[i][0:64, :], lambda i: k2_t[i][64:128, :],
                    )

                seq = [
                    (qg, h, kc)
                    for qg in range(n_qg)
                    for h in range(HPC)
                    for kc in range(nkc)
                ]
                pss_tiles = {}
                psu_tiles = {}

                def emit_scores(qg, h, kc):
                    qlo, qhi, klo, khi = head_views(h)
                    pss = pp.tile([128, qg_w], f32, tag="s", bufs=2, name="pss")
                    pss_tiles[(qg, h, kc)] = pss
                    ki, ko = (kc * 128) // QT, (kc * 128) % QT
                    for qi in range(nqi):
                        use_hi = pair and qi % 2 == 1
                        kv = khi(ki) if use_hi else klo(ki)
                        qv = qhi(qg * nqi + qi) if use_hi else qlo(qg * nqi + qi)
                        nc.tensor.matmul(
                            pss[:, bass.ts(qi, QT)],
                            kv[:, bass.ds(ko, 128)],
                            qv,
                            start=True, stop=True,
                            tile_position=(64, 0) if use_hi else ((0, 0) if pair else None),
                        )

                def emit_act_pv(qg, h, kc):
                    pss = pss_tiles.pop((qg, h, kc))
                    et = esb.tile([128, qg_w], bf16, tag="e")
                    if kc in dve_kcs:
                        nc.vector._custom_dve(
                            EXP2_OP, out=et.bitcast(mybir.dt.int16), in0=pss,
                            in1=eb1_sb, s0=EXP2_A128, s1=EXP2_MAGIC, imm2=128.0,
                        )
                    else:
                        nc.scalar.activation(
                            et, pss, Exp,
                            scale=float(np.log(2.0) / 128.0),
                            bias=abias_sb,
                        )
                    if kc == 0:
                        psu_tiles[(qg, h)] = [
                            pp.tile([DH + 1, QT], f32, tag=f"u{i}", bufs=2, name=f"psu{i}")
                            for i in range(nqi)
                        ]
                    psu = psu_tiles[(qg, h)]
                    for qi in range(nqi):
                        nc.tensor.matmul(
                            psu[qi],
                            v_t[kc][:, h, :],
                            et[:, bass.ts(qi, QT)],
                            start=(kc == 0), stop=(kc == nkc - 1),
                        )

                def emit_norm(qg, h):
                    psu = psu_tiles.pop((qg, h))
                    zt = zsb.tile([1, qg_w], f32, tag="z")
                    rz = zsb.tile([1, qg_w], f32, tag="r")
                    for qi in range(nqi):
                        nc.vector.tensor_scalar_sub(
                            zt[:, bass.ts(qi, QT)], psu[qi][DH : DH + 1, :], npad_sb
                        )
                    nc.vector.reciprocal(rz, zt)
                    rb = zsb.tile([DH, qg_w], f32, tag="rb")
                    rz_bcast = bass.AP(
                        tensor=rz.tensor, offset=rz.offset,
                        ap=[[1, 1], [0, DH], [1, qg_w]],
                    )
                    nc.sync.dma_start(out=rb, in_=rz_bcast)
                    for qi in range(nqi):
                        nc.vector.tensor_mul(
                            un_t[h][qg * nqi + qi][0:DH, :],
                            psu[qi][0:DH, :],
                            rb[:, bass.ts(qi, QT)],
                        )

                def emit_outproj_unit(qg, qi, ec):
                    qsl = bass.ts(qg * nqi + qi, QT)
                    po = pp.tile([128, qg_w], f32, tag="s", bufs=2, name="po")
                    for h in range(HPC):
                        nc.tensor.matmul(
                            po[:, 0:QT],
                            wo_sb[:, h, bass.ts(ec, 128)],
                            un_t[h][qg * nqi + qi][0:DH, :],
                            start=(h == 0), stop=(h == HPC - 1),
                        )
                    ob = obp.tile([128, QT], f32, tag="ob")
                    nc.vector.tensor_copy(ob, po[:, 0:QT])
                    nc.sync.dma_start(out=out[bass.ts(ec, 128), qsl], in_=ob)

                from collections import deque
                po_q = deque()
                with tc.high_priority(offset=prio):
                    emit_scores(*seq[0])
                    for t in range(len(seq)):
                        if t + 1 < len(seq):
                            emit_scores(*seq[t + 1])
                        qg, h, kc = seq[t]
                        emit_act_pv(qg, h, kc)
                        if po_q and t % 2 == 0:
                            emit_outproj_unit(*po_q.popleft())
                        if kc == nkc - 1:
                            emit_norm(qg, h)
                            if h == HPC - 1:
                                for qi in range(nqi):
                                    for ec in range(EC):
                                        po_q.append((qg, qi, ec))
                    while po_q:
                        emit_outproj_unit(*po_q.popleft())

        if repeat == 1:
            body()
        else:
            with tc.For_i(0, repeat, 1) as iv:
                body(iv)

    nc.finalize()
    return nc


# ---------------- host wrapper ----------------


def _prep_core_inputs(h, Wq, bq, Wk, bk, Wv, bv, Wo, att_mask, kept_idx, Skc):
    """Build in_maps for the 8 cores."""
    in_maps = []
    hT_b = []
    hTk_b = []
    npad_b = []
    for b in range(B):
        hb = np.asarray(h[b], np.float32)
        hT_b.append(np.ascontiguousarray(hb.T).astype(BF16))
        hk = hb[kept_idx[b]]  # [kept, D]
        pad = Skc - hk.shape[0]
        hkT = np.zeros((D, Skc), np.float32)
        hkT[:, : hk.shape[0]] = hk.T
        hTk_b.append(hkT.astype(BF16))
        npad_b.append(np.array([[float(pad) * EXP2_G]], np.float32))
    for c in range(NCORES):
        b, g = divmod(c, NCORES // B)
        hs = g * HPC * DH
        sl = slice(hs, hs + HPC * DH)
        in_maps.append(
            {
                "hT": hT_b[b],
                "hTk": hTk_b[b],
                "wq": np.ascontiguousarray(Wq[sl].T * SCORE_PRESCALE).astype(BF16),
                "wk": np.ascontiguousarray(Wk[sl].T).astype(BF16),
                "wv": np.ascontiguousarray(Wv[sl].T).astype(BF16),
                "wo": np.ascontiguousarray(Wo[:, sl].T).astype(BF16),
                "bq": (np.asarray(bq[sl], np.float32) * SCORE_PRESCALE).reshape(-1, 1),
                "bk": np.asarray(bk[sl], np.float32).reshape(-1, 1),
                "bv": np.asarray(bv[sl], BF16).reshape(1, -1),
                "npad": npad_b[b],
            }
        )
    return in_maps


def _reference_np(h, Wq, bq, Wk, bk, Wv, bv, Wo, bo, att_mask):
    """Numpy fallback (only used for degenerate masks)."""
    scale = 1.0 / np.sqrt(np.float32(DH))
    out = np.empty((B, S, D), np.float32)
    for b in range(B):
        q = (h[b] @ Wq.T + bq).reshape(S, H, DH).transpose(1, 0, 2)
        k = (h[b] @ Wk.T + bk).reshape(S, H, DH).transpose(1, 0, 2)
        v = (h[b] @ Wv.T + bv).reshape(S, H, DH).transpose(1, 0, 2)
        sc = np.einsum("hqd,hkd->hqk", q, k) * scale
        sc = np.where(att_mask[b][None, None, :] != 0, np.float32(-1e9), sc)
        sc -= sc.max(axis=-1, keepdims=True)
        e = np.exp(sc)
        p = e / e.sum(axis=-1, keepdims=True)
        att = np.einsum("hqk,hkd->hqd", p, v).transpose(1, 0, 2).reshape(S, H * DH)
        out[b] = att @ Wo.T + bo
    return out


def kernel(h, Wq, bq, Wk, bk, Wv, bv, Wo, bo, att_mask):
    h = np.asarray(h, np.float32)
    att_mask = np.asarray(att_mask)
    Wq, Wk, Wv, Wo = (np.asarray(x, np.float32) for x in (Wq, Wk, Wv, Wo))
    bq, bk, bv, bo = (np.asarray(x, np.float32) for x in (bq, bk, bv, bo))

    if np.abs(bk).max() > 0 or np.abs(bv).max() > 0:
        return _reference_np(h, Wq, bq, Wk, bk, Wv, bv, Wo, bo, att_mask)

    kept_idx = [np.nonzero(att_mask[b] == 0)[0] for b in range(B)]
    kept_max = max(len(k) for k in kept_idx)
    if kept_max == 0:
        return _reference_np(h, Wq, bq, Wk, bk, Wv, bv, Wo, bo, att_mask)
    Skc = max(128, ((kept_max + 127) // 128) * 128)

    if Skc not in _prog_cache:
        _prog_cache[Skc] = CachedRunner(build_program(Skc), NCORES)
    runner = _prog_cache[Skc]

    in_maps = _prep_core_inputs(h, Wq, bq, Wk, bk, Wv, bv, Wo, att_mask, kept_idx, Skc)
    results = runner(in_maps)

    out = np.empty((B, S, D), np.float32)
    for b in range(B):
        acc = np.zeros((D, S), np.float32)
        for g in range(NCORES // B):
            acc += results[b * (NCORES // B) + g]["out"]
        out[b] = acc.T + bo[None, :]
    return out



# revision 11
# speedup vs baseline: 1.3189x; 1.3189x over previous
"""Trainium2 Bass kernel for nn_MultiHeadAttention (B=2, S=4096, D=768, H=12, DH=64).

Sharding: 8 cores = 2 batches x 4 head-groups (3 heads each).
Each core computes its heads' attention for its batch and a partial
out^T = Wo_slice^T.T @ att^T ; host sums the 4 head-group partials per batch.

Mask trick: key positions with att_mask==1 are dropped on the host
(K/V computed only over kept positions, padded to a multiple of 128 with
zero columns). Pad columns give scores exactly 0 -> exp = 1, corrected by
Z -= n_pad. Pad V rows are zero so U is unaffected.
"""

import math

import numpy as np
import ml_dtypes

import concourse.bass as bass
import concourse.mybir as mybir
import concourse.tile as tile
from concourse import bacc
from concourse.bass_utils import run_bass_kernel_spmd

# ---------------- custom DVE exp2 op (bf16 bits of 2^(x/128)) ----------------
import concourse.dve_ops as _dve_ops
from concourse.dve_spec import (
    Spec as _Spec, Src0 as _Src0, C0 as _C0, C1 as _C1, C2 as _C2, C3 as _C3,
    Bin as _Bin, _spill_c3_to_src1 as _spill, lower as _dve_lower,
    _has_src1 as _dve_has_src1,
)
from concourse.dve_uop import AluOp as _AluOp, DveOpSpec as _DveOpSpec

EXP2_A128 = 44.234904699532095 / (128.0 * 128.0)
EXP2_B128 = -44.03494444066827 / 128.0
EXP2_MAGIC = float(np.float32(1.5 * 2.0**30))
EXP2_G = 1.9938757033212957  # effective output scale (trunc store, sim-measured)
SCORE_PRESCALE = 128.0 * np.log2(np.e) / 8.0  # folded into Wq/bq on host


def _exp2_ref(in0, in1, s0, s1, imm2):
    x = np.asarray(in0, np.float32)
    b = np.float32(np.asarray(in1).reshape(-1)[0]) if np.ndim(in1) else np.float32(in1)
    r = (x + np.float32(s1)) - np.float32(s1)
    F = np.abs(x - r)
    q = F * (np.float32(s0) * F + b)
    return q + (x + np.float32(imm2) * np.float32(imm2))


def _register_exp2_op():
    if "EXP2B_ANT" in _dve_ops._SUB_OPCODE_FOR_NAME:
        for _op in _dve_ops.OPS:
            if _op.name == "EXP2B_ANT":
                return _op
    K = _Bin(_AluOp.MULTIPLY, _C2, _C2)
    t = _Src0 + _C1
    r = t - _C1
    F = _Bin(_AluOp.ABSOLUTE_DIFF, _Src0, r)
    q = _Bin(_AluOp.MULTIPLY, F, _C0 * F + _C3)
    body = q + (_Src0 + K)
    spec = _Spec(body=_spill(body), reference=_exp2_ref)
    shas = {}
    for ver in ("v3", "v4"):
        try:
            tmp = _DveOpSpec(
                name="EXP2B_ANT", opcode=17, uops=_dve_lower(spec, ver=ver),
                rd1_en=_dve_has_src1(spec),
            )
            shas[ver] = tmp.sha(ver)
        except Exception:
            pass
    op = _dve_ops.DveOp("EXP2B_ANT", spec, False, shas)
    _dve_ops.OPS.append(op)
    _dve_ops.CUSTOM_DVE_SPECS[op.name] = op.spec
    _dve_ops._SUB_OPCODE_FOR_NAME[op.name] = 17
    return op


EXP2_OP = _register_exp2_op()


class CachedRunner:
    """Compile the Bass program into a PJRT executable once; reuse across calls."""

    def __init__(self, nc, n_cores=8):
        import jax
        from jax.sharding import Mesh, PartitionSpec
        from jax.experimental.shard_map import shard_map as _shard_map
        from concourse.bass2jax import (
            _bass_exec_p,
            install_neuronx_cc_hook,
            partition_id_tensor,
        )
        import concourse.mybir as _mybir

        install_neuronx_cc_hook()
        self.nc = nc
        self.n_cores = n_cores
        part_name = nc.partition_id_tensor.name if nc.partition_id_tensor else None
        in_names, out_names, out_avals, zero_shapes = [], [], [], []
        for alloc in nc.m.functions[0].allocations:
            if not isinstance(alloc, _mybir.MemoryLocationSet):
                continue
            name = alloc.memorylocations[0].name
            if alloc.kind == "ExternalInput":
                if name != part_name:
                    in_names.append(name)
            elif alloc.kind == "ExternalOutput":
                out_names.append(name)
                shape = tuple(alloc.tensor_shape)
                dtype = _mybir.dt.np(alloc.dtype)
                out_avals.append(jax.core.ShapedArray(shape, dtype))
                zero_shapes.append((shape, dtype))
        self.in_names, self.out_names = in_names, out_names
        self.out_avals = out_avals
        self.zero_shapes = zero_shapes
        n_params = len(in_names)
        all_in_names = tuple(in_names) + tuple(out_names)
        if part_name is not None:
            all_in_names = all_in_names + (part_name,)

        def _body(*args):
            operands = list(args)
            if part_name is not None:
                operands.append(partition_id_tensor())
            return tuple(
                _bass_exec_p.bind(
                    *operands,
                    out_avals=tuple(out_avals),
                    in_names=all_in_names,
                    out_names=tuple(out_names),
                    lowering_input_output_aliases=(),
                    sim_require_finite=True,
                    sim_require_nnan=True,
                    nc=nc,
                )
            )

        devices = jax.devices()[:n_cores]
        mesh = Mesh(np.asarray(devices), ("core",))
        nin = n_params + len(out_names)
        self._fn = jax.jit(
            _shard_map(
                _body,
                mesh=mesh,
                in_specs=(PartitionSpec("core"),) * nin,
                out_specs=(PartitionSpec("core"),) * len(out_names),
                check_rep=False,
            ),
            keep_unused=True,
        )
        self._jax = jax

    def __call__(self, in_maps):
        jax = self._jax
        concat = [
            np.concatenate([np.asarray(m[name]) for m in in_maps], axis=0)
            for name in self.in_names
        ]
        zeros = [
            np.zeros((self.n_cores * s[0],) + tuple(s[1:]), d)
            for s, d in self.zero_shapes
        ]
        outs = self._fn(*concat, *zeros)
        jax.block_until_ready(outs)
        res = []
        for c in range(self.n_cores):
            m = {}
            for i, name in enumerate(self.out_names):
                shape = self.out_avals[i].shape
                m[name] = np.asarray(outs[i]).reshape((self.n_cores,) + shape)[c]
            res.append(m)
        return res

B, S, D, H, DH = 2, 4096, 768, 12, 64
HPC = 3  # heads per core
NCORES = 8
EC = D // 128  # 6 e-chunks
QT = 512  # q tile (psum bank)
QG = 1024  # q group width for the exp ACT batch
BF16 = ml_dtypes.bfloat16

_prog_cache: dict = {}


def _kchunks(total, step):
    out = []
    o = 0
    while o < total:
        w = min(step, total - o)
        out.append((o, w))
        o += w
    return out


def build_program(Skc: int, s_full: int = S, repeat: int = 1, pair: bool = True, prio: int = 700):
    """Build the per-core Bass program. Skc = padded kept-key count (mult of 128)."""
    f32 = mybir.dt.float32
    bf16 = mybir.dt.bfloat16
    nkc = Skc // 128
    dve_kcs = {kc for kc in range(nkc) if kc % 5 in (1, 3) and kc < nkc - 2}
    n_qt = s_full // QT
    qg_w = min(QG, s_full)
    nqi = qg_w // QT
    n_qg = s_full // qg_w

    nc = bacc.Bacc()
    hT = nc.dram_tensor("hT", [D, s_full], bf16, kind="ExternalInput")
    hTk = nc.dram_tensor("hTk", [D, Skc], bf16, kind="ExternalInput")
    wq = nc.dram_tensor("wq", [D, HPC * DH], bf16, kind="ExternalInput")
    wk = nc.dram_tensor("wk", [D, HPC * DH], bf16, kind="ExternalInput")
    wv = nc.dram_tensor("wv", [D, HPC * DH], bf16, kind="ExternalInput")
    wo = nc.dram_tensor("wo", [HPC * DH, D], bf16, kind="ExternalInput")
    bq = nc.dram_tensor("bq", [HPC * DH, 1], f32, kind="ExternalInput")
    bk = nc.dram_tensor("bk", [HPC * DH, 1], f32, kind="ExternalInput")
    npad = nc.dram_tensor("npad", [1, 1], f32, kind="ExternalInput")
    out = nc.dram_tensor("out", [D, s_full], f32, kind="ExternalOutput")

    Exp = mybir.ActivationFunctionType.Exp

    import contextlib
    with tile.TileContext(nc) as tc, contextlib.ExitStack() as _stk:
        _pp = _stk.enter_context(tc.tile_pool(name="persist", bufs=1))
        def _tctile(shape, dtype, name):
            return _pp.tile(shape, dtype, name=name, tag=name)
        # ---- persistent SBUF tiles ----
        hT_sb = _tctile([128, EC, s_full], bf16, name="hT_sb")
        hTk_sb = _tctile([128, EC, Skc], bf16, name="hTk_sb")
        wq_sb = _tctile([128, EC, HPC * DH], bf16, name="wq_sb")
        wk_sb = _tctile([128, EC, HPC * DH], bf16, name="wk_sb")
        wv_sb = _tctile([128, EC, HPC * DH], bf16, name="wv_sb")
        wo_sb_t = _tctile([128, HPC, D], bf16, name="wo_sb")
        wo_sb = wo_sb_t[0:DH, :, :]
        bqp_sb = _tctile([128, 1], f32, name="bqp_sb")
        bq2_sb_t = _tctile([128, 1], f32, name="bq2_sb")
        bq2_sb = bq2_sb_t[0:DH, :]
        bkp_sb = _tctile([128, 1], f32, name="bkp_sb")
        bk2_sb_t = _tctile([128, 1], f32, name="bk2_sb")
        bk2_sb = bk2_sb_t[0:DH, :]
        eb1_sb = _tctile([128, 1], f32, name="eb1_sb")
        abias_sb = _tctile([128, 1], f32, name="abias_sb")
        # -npad at partition DH (row 64): bias for Ln(z - npad) on ScalarE
        nneg_sb_t = _tctile([128, 1], f32, name="nneg_sb")
        nneg_sb = nneg_sb_t[DH : DH + 1, :]

        qp_t = [_tctile([128, QT], bf16, name=f"qp{i}") for i in range(n_qt)]
        qp2_t = [_tctile([128, QT], bf16, name=f"qp2_{i}") for i in range(n_qt)]
        q2_t = [_tctile([128, QT], bf16, name=f"q2_{i}") for i in range(n_qt)]
        nkch = len(_kchunks(Skc, QT))
        kp_t = [_tctile([128, QT], bf16, name=f"kp{i}") for i in range(nkch)]
        kp2_t = [_tctile([128, QT], bf16, name=f"kp2_{i}") for i in range(nkch)]
        k2_t = [_tctile([128, QT], bf16, name=f"k2_{i}") for i in range(nkch)]
        v_t = [_tctile([128, HPC, DH + 1], bf16, name=f"v{sc}") for sc in range(nkc)]
        un_t = [
            [_tctile([128, QT], bf16, name=f"un{h}_{i}") for i in range(n_qt)]
            for h in range(HPC)
        ]

        # input DMAs (small weights first so projections can start immediately)
        nc.sync.dma_start(out=wq_sb, in_=wq[:, :].rearrange("(c p) m -> p c m", p=128))
        nc.sync.dma_start(out=wk_sb, in_=wk[:, :].rearrange("(c p) m -> p c m", p=128))
        nc.sync.dma_start(out=wv_sb, in_=wv[:, :].rearrange("(c p) m -> p c m", p=128))
        nc.sync.dma_start(out=wo_sb, in_=wo[:, :].rearrange("(h d) e -> d h e", d=DH))
        nc.sync.dma_start(out=bqp_sb, in_=bq[0:128, :])
        nc.sync.dma_start(out=bq2_sb, in_=bq[128 : HPC * DH, :])
        nc.sync.dma_start(out=bkp_sb, in_=bk[0:128, :])
        nc.sync.dma_start(out=bk2_sb, in_=bk[128 : HPC * DH, :])
        nc.sync.dma_start(out=nneg_sb, in_=npad[:, :])
        for ec in range(EC):
            nc.sync.dma_start(
                out=hT_sb[:, ec, :], in_=hT[bass.ts(ec, 128), :]
            )
            nc.sync.dma_start(
                out=hTk_sb[:, ec, :], in_=hTk[bass.ts(ec, 128), :]
            )
        nc.vector.memset(eb1_sb, EXP2_B128)
        nc.vector.memset(abias_sb, float(np.log(EXP2_G)))
        for sc in range(nkc):
            nc.vector.memset(v_t[sc][:, :, DH : DH + 1], 1.0)

        def body(_iv=None):
            import contextlib as _cl
            with _cl.ExitStack() as stk:
                pp = stk.enter_context(tc.tile_pool(name="ps", bufs=1, space="PSUM"))
                esb = stk.enter_context(tc.tile_pool(name="esb", bufs=3))
                zsb = stk.enter_context(tc.tile_pool(name="zsb", bufs=2))
                obp = stk.enter_context(tc.tile_pool(name="ob", bufs=3))

                # ---- projection emitters (psum slots shared with scores, tag "s") ----
                def q_proj(qi):
                    qsl = bass.ts(qi, QT)
                    ps = pp.tile([128, qg_w], f32, tag="s", bufs=2, name="ps")
                    for ec in range(EC):
                        nc.tensor.matmul(
                            ps[:, 0:QT], wq_sb[:, ec, 0:128], hT_sb[:, ec, qsl],
                            start=(ec == 0), stop=(ec == EC - 1),
                        )
                    for ec in range(EC):
                        nc.tensor.matmul(
                            ps[0:DH, QT : QT + QT], wq_sb[:, ec, 128 : HPC * DH],
                            hT_sb[:, ec, qsl],
                            start=(ec == 0), stop=(ec == EC - 1),
                        )
                    nc.vector.tensor_scalar_add(qp_t[qi][:, :], ps[:, 0:QT], bqp_sb)
                    nc.vector.tensor_scalar_add(q2_t[qi][0:DH, :], ps[0:DH, QT : QT + QT], bq2_sb)
                    nc.sync.dma_start(out=qp2_t[qi][0:64, :], in_=qp_t[qi][64:128, :])
                    nc.sync.dma_start(out=qp2_t[qi][64:128, :], in_=qp_t[qi][0:64, :])
                    nc.sync.dma_start(out=q2_t[qi][64:128, :], in_=q2_t[qi][0:64, :])

                def k_proj(i, off, w):
                    ksl = bass.ds(off, w)
                    ps = pp.tile([128, qg_w], f32, tag="s", bufs=2, name="ps")
                    for ec in range(EC):
                        nc.tensor.matmul(
                            ps[:, 0:w], wk_sb[:, ec, 0:128], hTk_sb[:, ec, ksl],
                            start=(ec == 0), stop=(ec == EC - 1),
                        )
                    for ec in range(EC):
                        nc.tensor.matmul(
                            ps[0:DH, QT : QT + w], wk_sb[:, ec, 128 : HPC * DH],
                            hTk_sb[:, ec, ksl],
                            start=(ec == 0), stop=(ec == EC - 1),
                        )
                    nc.vector.tensor_scalar_add(kp_t[i][:, 0:w], ps[:, 0:w], bkp_sb)
                    nc.vector.tensor_scalar_add(k2_t[i][0:DH, 0:w], ps[0:DH, QT : QT + w], bk2_sb)
                    nc.sync.dma_start(out=kp2_t[i][0:64, 0:w], in_=kp_t[i][64:128, 0:w])
                    nc.sync.dma_start(out=kp2_t[i][64:128, 0:w], in_=kp_t[i][0:64, 0:w])
                    nc.sync.dma_start(out=k2_t[i][64:128, 0:w], in_=k2_t[i][0:64, 0:w])

                def v_proj(sc):
                    psv = pp.tile([128, qg_w], f32, tag="s", bufs=2, name="psv")
                    for ec in range(EC):
                        nc.tensor.matmul(
                            psv[:, 0 : HPC * DH], hTk_sb[:, ec, bass.ts(sc, 128)],
                            wv_sb[:, ec, :],
                            start=(ec == 0), stop=(ec == EC - 1),
                        )
                    nc.vector.tensor_copy(
                        v_t[sc][:, :, 0:DH],
                        psv[:, 0 : HPC * DH].rearrange("p (h d) -> p h d", d=DH),
                    )

                # emit projections: enough for attention to start, rest follows
                for qi in range(nqi):
                    q_proj(qi)
                kch = _kchunks(Skc, QT)
                for i, (off, w) in enumerate(kch):
                    k_proj(i, off, w)
                    for sc in range(4 * i, min(4 * (i + 1), nkc)):
                        v_proj(sc)
                for sc in range(4 * len(kch), nkc):
                    v_proj(sc)
                for qi in range(nqi, n_qt):
                    q_proj(qi)

                # ---- flat pipelined attention stream ----
                def head_views(h):
                    if h == 0:
                        return (
                            lambda qi: qp_t[qi][0:64, :], lambda qi: qp2_t[qi][64:128, :],
                            lambda i: kp_t[i][0:64, :], lambda i: kp2_t[i][64:128, :],
                        )
                    if h == 1:
                        return (
                            lambda qi: qp2_t[qi][0:64, :], lambda qi: qp_t[qi][64:128, :],
                            lambda i: kp2_t[i][0:64, :], lambda i: kp_t[i][64:128, :],
                        )
                    return (
                        lambda qi: q2_t[qi][0:64, :], lambda qi: q2_t[qi][64:128, :],
                        lambda i: k2_t[i][0:64, :], lambda i: k2_t[i][64:128, :],
                    )

                seq = [
                    (qg, h, kc)
                    for qg in range(n_qg)
                    for h in range(HPC)
                    for kc in range(nkc)
                ]
                pss_tiles = {}
                psu_tiles = {}

                def emit_scores(qg, h, kc):
                    qlo, qhi, klo, khi = head_views(h)
                    pss = pp.tile([128, qg_w], f32, tag="s", bufs=2, name="pss")
                    pss_tiles[(qg, h, kc)] = pss
                    ki, ko = (kc * 128) // QT, (kc * 128) % QT
                    for qi in range(nqi):
                        use_hi = pair and qi % 2 == 1
                        kv = khi(ki) if use_hi else klo(ki)
                        qv = qhi(qg * nqi + qi) if use_hi else qlo(qg * nqi + qi)
                        nc.tensor.matmul(
                            pss[:, bass.ts(qi, QT)],
                            kv[:, bass.ds(ko, 128)],
                            qv,
                            start=True, stop=True,
                            tile_position=(64, 0) if use_hi else ((0, 0) if pair else None),
                        )

                def emit_act_pv(qg, h, kc):
                    pss = pss_tiles.pop((qg, h, kc))
                    et = esb.tile([128, qg_w], bf16, tag="e")
                    if kc in dve_kcs:
                        nc.vector._custom_dve(
                            EXP2_OP, out=et.bitcast(mybir.dt.int16), in0=pss,
                            in1=eb1_sb, s0=EXP2_A128, s1=EXP2_MAGIC, imm2=128.0,
                        )
                    else:
                        nc.scalar.activation(
                            et, pss, Exp,
                            scale=float(np.log(2.0) / 128.0),
                            bias=abias_sb,
                        )
                    if kc == 0:
                        psu_tiles[(qg, h)] = [
                            pp.tile([DH + 1, QT], f32, tag=f"u{i}", bufs=2, name=f"psu{i}")
                            for i in range(nqi)
                        ]
                    psu = psu_tiles[(qg, h)]
                    for qi in range(nqi):
                        nc.tensor.matmul(
                            psu[qi],
                            v_t[kc][:, h, :],
                            et[:, bass.ts(qi, QT)],
                            start=(kc == 0), stop=(kc == nkc - 1),
                        )

                def emit_norm(qg, h):
                    # 1/z = Exp(-Ln(z - npad)) on ScalarE: keeps the slow
                    # one-partition reciprocal off the (overloaded) DVE.
                    psu = psu_tiles.pop((qg, h))
                    Ln = mybir.ActivationFunctionType.Ln
                    lnz_t = zsb.tile([DH + 1, qg_w], f32, tag="z")
                    lnz = lnz_t[DH : DH + 1, :]
                    rz = zsb.tile([1, qg_w], f32, tag="r")
                    for qi in range(nqi):
                        nc.scalar.activation(
                            lnz[:, bass.ts(qi, QT)], psu[qi][DH : DH + 1, :],
                            Ln, bias=nneg_sb,
                        )
                    nc.scalar.activation(rz, lnz, Exp, scale=-1.0)
                    rb = zsb.tile([DH, qg_w], f32, tag="rb")
                    rz_bcast = bass.AP(
                        tensor=rz.tensor, offset=rz.offset,
                        ap=[[1, 1], [0, DH], [1, qg_w]],
                    )
                    nc.sync.dma_start(out=rb, in_=rz_bcast)
                    for qi in range(nqi):
                        nc.vector.tensor_mul(
                            un_t[h][qg * nqi + qi][0:DH, :],
                            psu[qi][0:DH, :],
                            rb[:, bass.ts(qi, QT)],
                        )

                def emit_outproj_unit(qg, qi, ec):
                    qsl = bass.ts(qg * nqi + qi, QT)
                    po = pp.tile([128, qg_w], f32, tag="s", bufs=2, name="po")
                    for h in range(HPC):
                        nc.tensor.matmul(
                            po[:, 0:QT],
                            wo_sb[:, h, bass.ts(ec, 128)],
                            un_t[h][qg * nqi + qi][0:DH, :],
                            start=(h == 0), stop=(h == HPC - 1),
                        )
                    ob = obp.tile([128, QT], f32, tag="ob")
                    nc.vector.tensor_copy(ob, po[:, 0:QT])
                    nc.sync.dma_start(out=out[bass.ts(ec, 128), qsl], in_=ob)

                from collections import deque
                po_q = deque()
                with tc.high_priority(offset=prio):
                    emit_scores(*seq[0])
                    for t in range(len(seq)):
                        if t + 1 < len(seq):
                            emit_scores(*seq[t + 1])
                        qg, h, kc = seq[t]
                        emit_act_pv(qg, h, kc)
                        if po_q and t % 2 == 0:
                            emit_outproj_unit(*po_q.popleft())
                        if kc == nkc - 1:
                            emit_norm(qg, h)
                            if h == HPC - 1:
                                for qi in range(nqi):
                                    for ec in range(EC):
                                        po_q.append((qg, qi, ec))
                    while po_q:
                        emit_outproj_unit(*po_q.popleft())

        if repeat == 1:
            body()
        else:
            with tc.For_i(0, repeat, 1) as iv:
                body(iv)

    nc.finalize()
    return nc


# ---------------- host wrapper ----------------


def _prep_core_inputs(h, Wq, bq, Wk, bk, Wv, bv, Wo, att_mask, kept_idx, Skc):
    """Build in_maps for the 8 cores."""
    in_maps = []
    hT_b = []
    hTk_b = []
    npad_b = []
    for b in range(B):
        hb = np.asarray(h[b], np.float32)
        hT_b.append(np.ascontiguousarray(hb.T).astype(BF16))
        hk = hb[kept_idx[b]]  # [kept, D]
        pad = Skc - hk.shape[0]
        hkT = np.zeros((D, Skc), np.float32)
        hkT[:, : hk.shape[0]] = hk.T
        hTk_b.append(hkT.astype(BF16))
        npad_b.append(np.array([[-float(pad) * EXP2_G]], np.float32))
    for c in range(NCORES):
        b, g = divmod(c, NCORES // B)
        hs = g * HPC * DH
        sl = slice(hs, hs + HPC * DH)
        in_maps.append(
            {
                "hT": hT_b[b],
                "hTk": hTk_b[b],
                "wq": np.ascontiguousarray(Wq[sl].T * SCORE_PRESCALE).astype(BF16),
                "wk": np.ascontiguousarray(Wk[sl].T).astype(BF16),
                "wv": np.ascontiguousarray(Wv[sl].T).astype(BF16),
                "wo": np.ascontiguousarray(Wo[:, sl].T).astype(BF16),
                "bq": (np.asarray(bq[sl], np.float32) * SCORE_PRESCALE).reshape(-1, 1),
                "bk": np.asarray(bk[sl], np.float32).reshape(-1, 1),
                "bv": np.asarray(bv[sl], BF16).reshape(1, -1),
                "npad": npad_b[b],
            }
        )
    return in_maps


def _reference_np(h, Wq, bq, Wk, bk, Wv, bv, Wo, bo, att_mask):
    """Numpy fallback (only used for degenerate masks)."""
    scale = 1.0 / np.sqrt(np.float32(DH))
    out = np.empty((B, S, D), np.float32)
    for b in range(B):
        q = (h[b] @ Wq.T + bq).reshape(S, H, DH).transpose(1, 0, 2)
        k = (h[b] @ Wk.T + bk).reshape(S, H, DH).transpose(1, 0, 2)
        v = (h[b] @ Wv.T + bv).reshape(S, H, DH).transpose(1, 0, 2)
        sc = np.einsum("hqd,hkd->hqk", q, k) * scale
        sc = np.where(att_mask[b][None, None, :] != 0, np.float32(-1e9), sc)
        sc -= sc.max(axis=-1, keepdims=True)
        e = np.exp(sc)
        p = e / e.sum(axis=-1, keepdims=True)
        att = np.einsum("hqk,hkd->hqd", p, v).transpose(1, 0, 2).reshape(S, H * DH)
        out[b] = att @ Wo.T + bo
    return out


def kernel(h, Wq, bq, Wk, bk, Wv, bv, Wo, bo, att_mask):
    h = np.asarray(h, np.float32)
    att_mask = np.asarray(att_mask)
    Wq, Wk, Wv, Wo = (np.asarray(x, np.float32) for x in (Wq, Wk, Wv, Wo))
    bq, bk, bv, bo = (np.asarray(x, np.float32) for x in (bq, bk, bv, bo))

    if np.abs(bk).max() > 0 or np.abs(bv).max() > 0:
        return _reference_np(h, Wq, bq, Wk, bk, Wv, bv, Wo, bo, att_mask)

    kept_idx = [np.nonzero(att_mask[b] == 0)[0] for b in range(B)]
    kept_max = max(len(k) for k in kept_idx)
    if kept_max == 0:
        return _reference_np(h, Wq, bq, Wk, bk, Wv, bv, Wo, bo, att_mask)
    Skc = max(128, ((kept_max + 127) // 128) * 128)

    if Skc not in _prog_cache:
        _prog_cache[Skc] = CachedRunner(build_program(Skc), NCORES)
    runner = _prog_cache[Skc]

    in_maps = _prep_core_inputs(h, Wq, bq, Wk, bk, Wv, bv, Wo, att_mask, kept_idx, Skc)
    results = runner(in_maps)

    out = np.empty((B, S, D), np.float32)
    for b in range(B):
        acc = np.zeros((D, S), np.float32)
        for g in range(NCORES // B):
            acc += results[b * (NCORES // B) + g]["out"]
        out[b] = acc.T + bo[None, :]
    return out



# revision 14
# speedup vs baseline: 1.3807x; 1.0468x over previous
"""Trainium2 Bass kernel for nn_MultiHeadAttention (B=2, S=4096, D=768, H=12, DH=64).

Sharding: 8 cores = 2 batches x 4 head-groups (3 heads each).
Each core computes its heads' attention for its batch and a partial
out^T = Wo_slice^T.T @ att^T ; host sums the 4 head-group partials per batch.

Mask trick: key positions with att_mask==1 are dropped on the host
(K/V computed only over kept positions, padded to a multiple of 128 with
zero columns). Pad columns give scores exactly 0 -> exp = 1, corrected by
Z -= n_pad. Pad V rows are zero so U is unaffected.
"""

import math

import numpy as np
import ml_dtypes

import concourse.bass as bass
import concourse.mybir as mybir
import concourse.tile as tile
from concourse import bacc
from concourse.bass_utils import run_bass_kernel_spmd

# ---------------- custom DVE exp2 op (bf16 bits of 2^(x/128)) ----------------
import concourse.dve_ops as _dve_ops
from concourse.dve_spec import (
    Spec as _Spec, Src0 as _Src0, C0 as _C0, C1 as _C1, C2 as _C2, C3 as _C3,
    Bin as _Bin, _spill_c3_to_src1 as _spill, lower as _dve_lower,
    _has_src1 as _dve_has_src1,
)
from concourse.dve_uop import AluOp as _AluOp, DveOpSpec as _DveOpSpec

EXP2_A128 = 44.234904699532095 / (128.0 * 128.0)
EXP2_B128 = -44.03494444066827 / 128.0
EXP2_MAGIC = float(np.float32(1.5 * 2.0**30))
EXP2_G = 1.9938757033212957  # effective output scale (trunc store, sim-measured)
SCORE_PRESCALE = 128.0 * np.log2(np.e) / 8.0  # folded into Wq/bq on host


def _exp2_ref(in0, in1, s0, s1, imm2):
    x = np.asarray(in0, np.float32)
    b = np.float32(np.asarray(in1).reshape(-1)[0]) if np.ndim(in1) else np.float32(in1)
    r = (x + np.float32(s1)) - np.float32(s1)
    F = np.abs(x - r)
    q = F * (np.float32(s0) * F + b)
    return q + (x + np.float32(imm2) * np.float32(imm2))


def _register_exp2_op():
    if "EXP2B_ANT" in _dve_ops._SUB_OPCODE_FOR_NAME:
        for _op in _dve_ops.OPS:
            if _op.name == "EXP2B_ANT":
                return _op
    K = _Bin(_AluOp.MULTIPLY, _C2, _C2)
    t = _Src0 + _C1
    r = t - _C1
    F = _Bin(_AluOp.ABSOLUTE_DIFF, _Src0, r)
    q = _Bin(_AluOp.MULTIPLY, F, _C0 * F + _C3)
    body = q + (_Src0 + K)
    spec = _Spec(body=_spill(body), reference=_exp2_ref)
    shas = {}
    for ver in ("v3", "v4"):
        try:
            tmp = _DveOpSpec(
                name="EXP2B_ANT", opcode=17, uops=_dve_lower(spec, ver=ver),
                rd1_en=_dve_has_src1(spec),
            )
            shas[ver] = tmp.sha(ver)
        except Exception:
            pass
    op = _dve_ops.DveOp("EXP2B_ANT", spec, False, shas)
    _dve_ops.OPS.append(op)
    _dve_ops.CUSTOM_DVE_SPECS[op.name] = op.spec
    _dve_ops._SUB_OPCODE_FOR_NAME[op.name] = 17
    return op


EXP2_OP = _register_exp2_op()


class CachedRunner:
    """Compile the Bass program into a PJRT executable once; reuse across calls."""

    def __init__(self, nc, n_cores=8):
        import jax
        from jax.sharding import Mesh, PartitionSpec
        from jax.experimental.shard_map import shard_map as _shard_map
        from concourse.bass2jax import (
            _bass_exec_p,
            install_neuronx_cc_hook,
            partition_id_tensor,
        )
        import concourse.mybir as _mybir

        install_neuronx_cc_hook()
        self.nc = nc
        self.n_cores = n_cores
        part_name = nc.partition_id_tensor.name if nc.partition_id_tensor else None
        in_names, out_names, out_avals, zero_shapes = [], [], [], []
        for alloc in nc.m.functions[0].allocations:
            if not isinstance(alloc, _mybir.MemoryLocationSet):
                continue
            name = alloc.memorylocations[0].name
            if alloc.kind == "ExternalInput":
                if name != part_name:
                    in_names.append(name)
            elif alloc.kind == "ExternalOutput":
                out_names.append(name)
                shape = tuple(alloc.tensor_shape)
                dtype = _mybir.dt.np(alloc.dtype)
                out_avals.append(jax.core.ShapedArray(shape, dtype))
                zero_shapes.append((shape, dtype))
        self.in_names, self.out_names = in_names, out_names
        self.out_avals = out_avals
        self.zero_shapes = zero_shapes
        n_params = len(in_names)
        all_in_names = tuple(in_names) + tuple(out_names)
        if part_name is not None:
            all_in_names = all_in_names + (part_name,)

        def _body(*args):
            operands = list(args)
            if part_name is not None:
                operands.append(partition_id_tensor())
            return tuple(
                _bass_exec_p.bind(
                    *operands,
                    out_avals=tuple(out_avals),
                    in_names=all_in_names,
                    out_names=tuple(out_names),
                    lowering_input_output_aliases=(),
                    sim_require_finite=True,
                    sim_require_nnan=True,
                    nc=nc,
                )
            )

        devices = jax.devices()[:n_cores]
        mesh = Mesh(np.asarray(devices), ("core",))
        nin = n_params + len(out_names)
        self._fn = jax.jit(
            _shard_map(
                _body,
                mesh=mesh,
                in_specs=(PartitionSpec("core"),) * nin,
                out_specs=(PartitionSpec("core"),) * len(out_names),
                check_rep=False,
            ),
            keep_unused=True,
        )
        self._jax = jax

    def __call__(self, in_maps):
        jax = self._jax
        concat = [
            np.concatenate([np.asarray(m[name]) for m in in_maps], axis=0)
            for name in self.in_names
        ]
        zeros = [
            np.zeros((self.n_cores * s[0],) + tuple(s[1:]), d)
            for s, d in self.zero_shapes
        ]
        outs = self._fn(*concat, *zeros)
        jax.block_until_ready(outs)
        res = []
        for c in range(self.n_cores):
            m = {}
            for i, name in enumerate(self.out_names):
                shape = self.out_avals[i].shape
                m[name] = np.asarray(outs[i]).reshape((self.n_cores,) + shape)[c]
            res.append(m)
        return res

B, S, D, H, DH = 2, 4096, 768, 12, 64
HPC = 3  # heads per core
NCORES = 8
EC = D // 128  # 6 e-chunks
QT = 512  # q tile (psum bank)
QG = 1024  # q group width for the exp ACT batch
BF16 = ml_dtypes.bfloat16

_prog_cache: dict = {}


def _kchunks(total, step):
    out = []
    o = 0
    while o < total:
        w = min(step, total - o)
        out.append((o, w))
        o += w
    return out


def _force_single_act_table(nc):
    """Steer the act-table-load pass so Exp AND Ln both resolve to the
    natural_log_exp_and_others set (one resident table, no thrashing).
    Indices into the table list are preserved, so walrus sees consistent
    act_func_set_ids."""
    import types

    import bass_rust
    from concourse.hw_specs import get_activation_tables

    both = {mybir.ActivationFunctionType.Exp, mybir.ActivationFunctionType.Ln}

    def _patched(self):
        has_activation = any(
            isinstance(i, mybir.InstActivation)
            for b in self.main_func.blocks
            for i in b.instructions
        )
        if not has_activation:
            return
        tables = []
        for name, funcs in get_activation_tables(self.m.arch).items():
            if name != "natural_log_exp_and_others":
                funcs = funcs - both
            tables.append((name, funcs))
        bass_rust.insert_act_table_loads(self, tables)

    nc.insert_act_table_loads = types.MethodType(_patched, nc)


def build_program(Skc: int, s_full: int = S, repeat: int = 1, pair: bool = True, prio: int = 700):
    """Build the per-core Bass program. Skc = padded kept-key count (mult of 128)."""
    f32 = mybir.dt.float32
    bf16 = mybir.dt.bfloat16
    nkc = Skc // 128
    dve_kcs = {kc for kc in range(nkc) if kc % 5 in (1, 3) and kc < nkc - 2}
    n_qt = s_full // QT
    qg_w = min(QG, s_full)
    nqi = qg_w // QT
    n_qg = s_full // qg_w

    nc = bacc.Bacc()
    _force_single_act_table(nc)
    hT = nc.dram_tensor("hT", [D, s_full], bf16, kind="ExternalInput")
    hTk = nc.dram_tensor("hTk", [D, Skc], bf16, kind="ExternalInput")
    wq = nc.dram_tensor("wq", [D, HPC * DH], bf16, kind="ExternalInput")
    wk = nc.dram_tensor("wk", [D, HPC * DH], bf16, kind="ExternalInput")
    wv = nc.dram_tensor("wv", [D, HPC * DH], bf16, kind="ExternalInput")
    wo = nc.dram_tensor("wo", [HPC * DH, D], bf16, kind="ExternalInput")
    bq = nc.dram_tensor("bq", [HPC * DH, 1], f32, kind="ExternalInput")
    bk = nc.dram_tensor("bk", [HPC * DH, 1], f32, kind="ExternalInput")
    npad = nc.dram_tensor("npad", [1, 1], f32, kind="ExternalInput")
    out = nc.dram_tensor("out", [D, s_full], f32, kind="ExternalOutput")

    Exp = mybir.ActivationFunctionType.Exp

    import contextlib
    with tile.TileContext(nc) as tc, contextlib.ExitStack() as _stk:
        _pp = _stk.enter_context(tc.tile_pool(name="persist", bufs=1))
        def _tctile(shape, dtype, name):
            return _pp.tile(shape, dtype, name=name, tag=name)
        # ---- persistent SBUF tiles ----
        hT_sb = _tctile([128, EC, s_full], bf16, name="hT_sb")
        hTk_sb = _tctile([128, EC, Skc], bf16, name="hTk_sb")
        wq_sb = _tctile([128, EC, HPC * DH], bf16, name="wq_sb")
        wk_sb = _tctile([128, EC, HPC * DH], bf16, name="wk_sb")
        wv_sb = _tctile([128, EC, HPC * DH], bf16, name="wv_sb")
        wo_sb_t = _tctile([128, HPC, D], bf16, name="wo_sb")
        wo_sb = wo_sb_t[0:DH, :, :]
        bqp_sb = _tctile([128, 1], f32, name="bqp_sb")
        bq2_sb_t = _tctile([128, 1], f32, name="bq2_sb")
        bq2_sb = bq2_sb_t[0:DH, :]
        bkp_sb = _tctile([128, 1], f32, name="bkp_sb")
        bk2_sb_t = _tctile([128, 1], f32, name="bk2_sb")
        bk2_sb = bk2_sb_t[0:DH, :]
        eb1_sb = _tctile([128, 1], f32, name="eb1_sb")
        abias_sb = _tctile([128, 1], f32, name="abias_sb")
        # -npad at partition DH (row 64): bias for Ln(z - npad) on ScalarE
        nneg_sb_t = _tctile([128, 1], f32, name="nneg_sb")
        nneg_sb = nneg_sb_t[DH : DH + 1, :]

        qp_t = [_tctile([128, QT], bf16, name=f"qp{i}") for i in range(n_qt)]
        qp2_t = [_tctile([128, QT], bf16, name=f"qp2_{i}") for i in range(n_qt)]
        q2_t = [_tctile([128, QT], bf16, name=f"q2_{i}") for i in range(n_qt)]
        nkch = len(_kchunks(Skc, QT))
        kp_t = [_tctile([128, QT], bf16, name=f"kp{i}") for i in range(nkch)]
        kp2_t = [_tctile([128, QT], bf16, name=f"kp2_{i}") for i in range(nkch)]
        k2_t = [_tctile([128, QT], bf16, name=f"k2_{i}") for i in range(nkch)]
        v_t = [_tctile([128, HPC, DH + 1], bf16, name=f"v{sc}") for sc in range(nkc)]
        un_t = [
            [_tctile([128, QT], bf16, name=f"un{h}_{i}") for i in range(n_qt)]
            for h in range(HPC)
        ]

        # input DMAs (small weights first so projections can start immediately)
        nc.sync.dma_start(out=wq_sb, in_=wq[:, :].rearrange("(c p) m -> p c m", p=128))
        nc.sync.dma_start(out=wk_sb, in_=wk[:, :].rearrange("(c p) m -> p c m", p=128))
        nc.sync.dma_start(out=wv_sb, in_=wv[:, :].rearrange("(c p) m -> p c m", p=128))
        nc.sync.dma_start(out=wo_sb, in_=wo[:, :].rearrange("(h d) e -> d h e", d=DH))
        nc.sync.dma_start(out=bqp_sb, in_=bq[0:128, :])
        nc.sync.dma_start(out=bq2_sb, in_=bq[128 : HPC * DH, :])
        nc.sync.dma_start(out=bkp_sb, in_=bk[0:128, :])
        nc.sync.dma_start(out=bk2_sb, in_=bk[128 : HPC * DH, :])
        nc.sync.dma_start(out=nneg_sb, in_=npad[:, :])
        for ec in range(EC):
            nc.sync.dma_start(
                out=hT_sb[:, ec, :], in_=hT[bass.ts(ec, 128), :]
            )
            nc.sync.dma_start(
                out=hTk_sb[:, ec, :], in_=hTk[bass.ts(ec, 128), :]
            )
        nc.vector.memset(eb1_sb, EXP2_B128)
        nc.vector.memset(abias_sb, float(np.log(EXP2_G)))
        for sc in range(nkc):
            nc.vector.memset(v_t[sc][:, :, DH : DH + 1], 1.0)

        def body(_iv=None):
            import contextlib as _cl
            with _cl.ExitStack() as stk:
                pp = stk.enter_context(tc.tile_pool(name="ps", bufs=1, space="PSUM"))
                esb = stk.enter_context(tc.tile_pool(name="esb", bufs=3))
                zsb = stk.enter_context(tc.tile_pool(name="zsb", bufs=2))
                obp = stk.enter_context(tc.tile_pool(name="ob", bufs=3))

                # ---- projection emitters (psum slots shared with scores, tag "s") ----
                def q_proj(qi):
                    qsl = bass.ts(qi, QT)
                    ps = pp.tile([128, qg_w], f32, tag="s", bufs=2, name="ps")
                    for ec in range(EC):
                        nc.tensor.matmul(
                            ps[:, 0:QT], wq_sb[:, ec, 0:128], hT_sb[:, ec, qsl],
                            start=(ec == 0), stop=(ec == EC - 1),
                        )
                    for ec in range(EC):
                        nc.tensor.matmul(
                            ps[0:DH, QT : QT + QT], wq_sb[:, ec, 128 : HPC * DH],
                            hT_sb[:, ec, qsl],
                            start=(ec == 0), stop=(ec == EC - 1),
                        )
                    nc.vector.tensor_scalar_add(qp_t[qi][:, :], ps[:, 0:QT], bqp_sb)
                    nc.vector.tensor_scalar_add(q2_t[qi][0:DH, :], ps[0:DH, QT : QT + QT], bq2_sb)
                    nc.sync.dma_start(out=qp2_t[qi][0:64, :], in_=qp_t[qi][64:128, :])
                    nc.sync.dma_start(out=qp2_t[qi][64:128, :], in_=qp_t[qi][0:64, :])
                    nc.sync.dma_start(out=q2_t[qi][64:128, :], in_=q2_t[qi][0:64, :])

                def k_proj(i, off, w):
                    ksl = bass.ds(off, w)
                    ps = pp.tile([128, qg_w], f32, tag="s", bufs=2, name="ps")
                    for ec in range(EC):
                        nc.tensor.matmul(
                            ps[:, 0:w], wk_sb[:, ec, 0:128], hTk_sb[:, ec, ksl],
                            start=(ec == 0), stop=(ec == EC - 1),
                        )
                    for ec in range(EC):
                        nc.tensor.matmul(
                            ps[0:DH, QT : QT + w], wk_sb[:, ec, 128 : HPC * DH],
                            hTk_sb[:, ec, ksl],
                            start=(ec == 0), stop=(ec == EC - 1),
                        )
                    nc.vector.tensor_scalar_add(kp_t[i][:, 0:w], ps[:, 0:w], bkp_sb)
                    nc.vector.tensor_scalar_add(k2_t[i][0:DH, 0:w], ps[0:DH, QT : QT + w], bk2_sb)
                    nc.sync.dma_start(out=kp2_t[i][0:64, 0:w], in_=kp_t[i][64:128, 0:w])
                    nc.sync.dma_start(out=kp2_t[i][64:128, 0:w], in_=kp_t[i][0:64, 0:w])
                    nc.sync.dma_start(out=k2_t[i][64:128, 0:w], in_=k2_t[i][0:64, 0:w])

                def v_proj(sc):
                    psv = pp.tile([128, qg_w], f32, tag="s", bufs=2, name="psv")
                    for ec in range(EC):
                        nc.tensor.matmul(
                            psv[:, 0 : HPC * DH], hTk_sb[:, ec, bass.ts(sc, 128)],
                            wv_sb[:, ec, :],
                            start=(ec == 0), stop=(ec == EC - 1),
                        )
                    nc.vector.tensor_copy(
                        v_t[sc][:, :, 0:DH],
                        psv[:, 0 : HPC * DH].rearrange("p (h d) -> p h d", d=DH),
                    )

                # emit projections: enough for attention to start, rest follows
                for qi in range(nqi):
                    q_proj(qi)
                kch = _kchunks(Skc, QT)
                for i, (off, w) in enumerate(kch):
                    k_proj(i, off, w)
                    for sc in range(4 * i, min(4 * (i + 1), nkc)):
                        v_proj(sc)
                for sc in range(4 * len(kch), nkc):
                    v_proj(sc)
                for qi in range(nqi, n_qt):
                    q_proj(qi)

                # ---- flat pipelined attention stream ----
                def head_views(h):
                    if h == 0:
                        return (
                            lambda qi: qp_t[qi][0:64, :], lambda qi: qp2_t[qi][64:128, :],
                            lambda i: kp_t[i][0:64, :], lambda i: kp2_t[i][64:128, :],
                        )
                    if h == 1:
                        return (
                            lambda qi: qp2_t[qi][0:64, :], lambda qi: qp_t[qi][64:128, :],
                            lambda i: kp2_t[i][0:64, :], lambda i: kp_t[i][64:128, :],
                        )
                    return (
                        lambda qi: q2_t[qi][0:64, :], lambda qi: q2_t[qi][64:128, :],
                        lambda i: k2_t[i][0:64, :], lambda i: k2_t[i][64:128, :],
                    )

                seq = [
                    (qg, h, kc)
                    for qg in range(n_qg)
                    for h in range(HPC)
                    for kc in range(nkc)
                ]
                pss_tiles = {}
                psu_tiles = {}

                def emit_scores(qg, h, kc):
                    qlo, qhi, klo, khi = head_views(h)
                    pss = pp.tile([128, qg_w], f32, tag="s", bufs=2, name="pss")
                    pss_tiles[(qg, h, kc)] = pss
                    ki, ko = (kc * 128) // QT, (kc * 128) % QT
                    for qi in range(nqi):
                        use_hi = pair and qi % 2 == 1
                        kv = khi(ki) if use_hi else klo(ki)
                        qv = qhi(qg * nqi + qi) if use_hi else qlo(qg * nqi + qi)
                        nc.tensor.matmul(
                            pss[:, bass.ts(qi, QT)],
                            kv[:, bass.ds(ko, 128)],
                            qv,
                            start=True, stop=True,
                            tile_position=(64, 0) if use_hi else ((0, 0) if pair else None),
                        )

                def emit_act_pv(qg, h, kc):
                    pss = pss_tiles.pop((qg, h, kc))
                    et = esb.tile([128, qg_w], bf16, tag="e")
                    if kc in dve_kcs:
                        nc.vector._custom_dve(
                            EXP2_OP, out=et.bitcast(mybir.dt.int16), in0=pss,
                            in1=eb1_sb, s0=EXP2_A128, s1=EXP2_MAGIC, imm2=128.0,
                        )
                    else:
                        nc.scalar.activation(
                            et, pss, Exp,
                            scale=float(np.log(2.0) / 128.0),
                            bias=abias_sb,
                        )
                    if kc == 0:
                        psu_tiles[(qg, h)] = [
                            pp.tile([DH + 1, QT], f32, tag=f"u{i}", bufs=2, name=f"psu{i}")
                            for i in range(nqi)
                        ]
                    psu = psu_tiles[(qg, h)]
                    for qi in range(nqi):
                        nc.tensor.matmul(
                            psu[qi],
                            v_t[kc][:, h, :],
                            et[:, bass.ts(qi, QT)],
                            start=(kc == 0), stop=(kc == nkc - 1),
                        )

                def emit_norm(qg, h):
                    # 1/z = Exp(-Ln(z - npad)) on ScalarE: keeps the slow
                    # one-partition reciprocal off the (overloaded) DVE.
                    psu = psu_tiles.pop((qg, h))
                    Ln = mybir.ActivationFunctionType.Ln
                    lnz_t = zsb.tile([DH + 1, qg_w], f32, tag="z")
                    lnz = lnz_t[DH : DH + 1, :]
                    rz = zsb.tile([1, qg_w], f32, tag="r")
                    for qi in range(nqi):
                        nc.scalar.activation(
                            lnz[:, bass.ts(qi, QT)], psu[qi][DH : DH + 1, :],
                            Ln, bias=nneg_sb,
                        )
                    nc.scalar.activation(rz, lnz, Exp, scale=-1.0)
                    rb = zsb.tile([DH, qg_w], f32, tag="rb")
                    rz_bcast = bass.AP(
                        tensor=rz.tensor, offset=rz.offset,
                        ap=[[1, 1], [0, DH], [1, qg_w]],
                    )
                    nc.sync.dma_start(out=rb, in_=rz_bcast)
                    for qi in range(nqi):
                        nc.vector.tensor_mul(
                            un_t[h][qg * nqi + qi][0:DH, :],
                            psu[qi][0:DH, :],
                            rb[:, bass.ts(qi, QT)],
                        )

                def emit_outproj_unit(qg, qi, ec):
                    qsl = bass.ts(qg * nqi + qi, QT)
                    po = pp.tile([128, qg_w], f32, tag="s", bufs=2, name="po")
                    for h in range(HPC):
                        nc.tensor.matmul(
                            po[:, 0:QT],
                            wo_sb[:, h, bass.ts(ec, 128)],
                            un_t[h][qg * nqi + qi][0:DH, :],
                            start=(h == 0), stop=(h == HPC - 1),
                        )
                    ob = obp.tile([128, QT], f32, tag="ob")
                    nc.vector.tensor_copy(ob, po[:, 0:QT])
                    nc.sync.dma_start(out=out[bass.ts(ec, 128), qsl], in_=ob)

                from collections import deque
                po_q = deque()
                PO_DELAY = 6  # steps of slack so un (z-chain) is ready before
                # the out-proj matmul enters the PE FIFO (avoids HOL blocking)
                with tc.high_priority(offset=prio):
                    emit_scores(*seq[0])
                    for t in range(len(seq)):
                        if t + 1 < len(seq):
                            emit_scores(*seq[t + 1])
                        qg, h, kc = seq[t]
                        emit_act_pv(qg, h, kc)
                        if po_q and t % 2 == 0 and t - po_q[0][0] >= PO_DELAY:
                            emit_outproj_unit(*po_q.popleft()[1])
                        if kc == nkc - 1:
                            emit_norm(qg, h)
                            if h == HPC - 1:
                                for qi in range(nqi):
                                    for ec in range(EC):
                                        po_q.append((t, (qg, qi, ec)))
                    while po_q:
                        emit_outproj_unit(*po_q.popleft()[1])

        if repeat == 1:
            body()
        else:
            with tc.For_i(0, repeat, 1) as iv:
                body(iv)

    nc.finalize()
    return nc


# ---------------- host wrapper ----------------


def _prep_core_inputs(h, Wq, bq, Wk, bk, Wv, bv, Wo, att_mask, kept_idx, Skc):
    """Build in_maps for the 8 cores."""
    in_maps = []
    hT_b = []
    hTk_b = []
    npad_b = []
    for b in range(B):
        hb = np.asarray(h[b], np.float32)
        hT_b.append(np.ascontiguousarray(hb.T).astype(BF16))
        hk = hb[kept_idx[b]]  # [kept, D]
        pad = Skc - hk.shape[0]
        hkT = np.zeros((D, Skc), np.float32)
        hkT[:, : hk.shape[0]] = hk.T
        hTk_b.append(hkT.astype(BF16))
        npad_b.append(np.array([[-float(pad) * EXP2_G]], np.float32))
    for c in range(NCORES):
        b, g = divmod(c, NCORES // B)
        hs = g * HPC * DH
        sl = slice(hs, hs + HPC * DH)
        in_maps.append(
            {
                "hT": hT_b[b],
                "hTk": hTk_b[b],
                "wq": np.ascontiguousarray(Wq[sl].T * SCORE_PRESCALE).astype(BF16),
                "wk": np.ascontiguousarray(Wk[sl].T).astype(BF16),
                "wv": np.ascontiguousarray(Wv[sl].T).astype(BF16),
                "wo": np.ascontiguousarray(Wo[:, sl].T).astype(BF16),
                "bq": (np.asarray(bq[sl], np.float32) * SCORE_PRESCALE).reshape(-1, 1),
                "bk": np.asarray(bk[sl], np.float32).reshape(-1, 1),
                "bv": np.asarray(bv[sl], BF16).reshape(1, -1),
                "npad": npad_b[b],
            }
        )
    return in_maps


def _reference_np(h, Wq, bq, Wk, bk, Wv, bv, Wo, bo, att_mask):
    """Numpy fallback (only used for degenerate masks)."""
    scale = 1.0 / np.sqrt(np.float32(DH))
    out = np.empty((B, S, D), np.float32)
    for b in range(B):
        q = (h[b] @ Wq.T + bq).reshape(S, H, DH).transpose(1, 0, 2)
        k = (h[b] @ Wk.T + bk).reshape(S, H, DH).transpose(1, 0, 2)
        v = (h[b] @ Wv.T + bv).reshape(S, H, DH).transpose(1, 0, 2)
        sc = np.einsum("hqd,hkd->hqk", q, k) * scale
        sc = np.where(att_mask[b][None, None, :] != 0, np.float32(-1e9), sc)
        sc -= sc.max(axis=-1, keepdims=True)
        e = np.exp(sc)
        p = e / e.sum(axis=-1, keepdims=True)
        att = np.einsum("hqk,hkd->hqd", p, v).transpose(1, 0, 2).reshape(S, H * DH)
        out[b] = att @ Wo.T + bo
    return out


def kernel(h, Wq, bq, Wk, bk, Wv, bv, Wo, bo, att_mask):
    h = np.asarray(h, np.float32)
    att_mask = np.asarray(att_mask)
    Wq, Wk, Wv, Wo = (np.asarray(x, np.float32) for x in (Wq, Wk, Wv, Wo))
    bq, bk, bv, bo = (np.asarray(x, np.float32) for x in (bq, bk, bv, bo))

    if np.abs(bk).max() > 0 or np.abs(bv).max() > 0:
        return _reference_np(h, Wq, bq, Wk, bk, Wv, bv, Wo, bo, att_mask)

    kept_idx = [np.nonzero(att_mask[b] == 0)[0] for b in range(B)]
    kept_max = max(len(k) for k in kept_idx)
    if kept_max == 0:
        return _reference_np(h, Wq, bq, Wk, bk, Wv, bv, Wo, bo, att_mask)
    Skc = max(128, ((kept_max + 127) // 128) * 128)

    if Skc not in _prog_cache:
        _prog_cache[Skc] = CachedRunner(build_program(Skc), NCORES)
    runner = _prog_cache[Skc]

    in_maps = _prep_core_inputs(h, Wq, bq, Wk, bk, Wv, bv, Wo, att_mask, kept_idx, Skc)
    results = runner(in_maps)

    out = np.empty((B, S, D), np.float32)
    for b in range(B):
        acc = np.zeros((D, S), np.float32)
        for g in range(NCORES // B):
            acc += results[b * (NCORES // B) + g]["out"]
        out[b] = acc.T + bo[None, :]
    return out

